# revision 17
# baseline (speedup 1.0000x reference)
"""Distance-biased FAVOR+ fast attention on 8 Trainium2 NeuronCores.

Strategy: shard the 32 (batch, head) pairs across 8 cores (4 pairs/core).
Per pair, the device computes:
    qs'^T = exp(W~ @ Xq~^T)    (transposed layout, m on partitions)  [bf16]
    ks'   = exp(Xk~ @ W~^T)    (natural layout, L on partitions)     [bf16]
    buf1  = ks'^T @ c          (PSUM accumulation over 32 L-chunks)
    buf2  = qs' @ buf1         (natural layout), then num / den
where Xq~/Xk~ are host-precomputed 73-dim augmented features
  [x*D^-0.25 (64), fourier dist features (8), h = 0.5*|x|^2 + ln(16)]
padded to 128 rows (full-partition DMAs run 5x faster than 73-row ones)
in fp16, and W~ = [qk_proj | -1] so exp(W~ X~^T) = exp(x W^T - h)/16 = phi(x).

The exp is split across two engines: ACT does exact exp on the q side +
8 k-chunks; the DVE computes the remaining 24 k-chunks with a Schraudolph
bitcast exp (bf16 bits = int16(logit*128/ln2 + 127*128 - C)), one
tensor_scalar mult-add per group. The +-3% ripple of that approximation
averages out through buf1/buf2; measured end-to-end rel err ~1.2e-2
(gate 2e-2). The /sqrt(m) factor and the EPS clamp on den cancel /
are dead code respectively (den >= 2e-4 on this distribution).
"""

import numpy as np
import ml_dtypes

B, L, H, D, DV = 4, 4096, 8, 64, 64
M = 256
DSH = 8             # per-head fourier feature dim (S // H)
KAUG = D + DSH + 1  # 73
NCORES = 8
PPC = (B * H) // NCORES  # pairs per core = 4
NCH = L // 128           # 32 chunks of 128 rows
EPS = 1e-6
LN16 = float(np.log(16.0))
TWO_PI = 2.0 * np.pi
NORM = float(D) ** -0.25

# Schraudolph bitcast-exp constants (bf16/int16 variant)
SCH_A = 128.0 / float(np.log(2.0))
SCH_C = 9.0
SCH_B = 127.0 * 128.0 - SCH_C
# k-chunk groups of 2 (512-f32 = one psum bank each): DVE Schraudolph
# groups first (they pace buf1), exact-exp ACT groups last
K_GROUPS = (("dve",) * 13) + (("act",) * 3)

_CACHE = {}


def _build_kernel():
    import concourse.bass as bass
    import concourse.bacc as bacc
    import concourse.mybir as mybir
    import concourse.tile as tile

    f32 = mybir.dt.float32
    bf16 = mybir.dt.bfloat16
    fp16 = mybir.dt.float16
    i16 = mybir.dt.int16
    Exp = mybir.ActivationFunctionType.Exp
    mult = mybir.AluOpType.mult
    add = mybir.AluOpType.add

    nc = bacc.Bacc("TRN2", debug=False, num_devices=NCORES)
    xq_t = nc.dram_tensor("xq_t", [PPC, 128, L], fp16, kind="ExternalInput")
    xk_t = nc.dram_tensor("xk_t", [PPC, 128, L], fp16, kind="ExternalInput")
    cb = nc.dram_tensor("cb", [PPC, 128, NCH, DV + 1], bf16, kind="ExternalInput")
    wt = nc.dram_tensor("wt", [128, M], fp16, kind="ExternalInput")
    out = nc.dram_tensor("out", [PPC, 128, NCH, DV], fp16, kind="ExternalOutput")

    with tile.TileContext(nc) as tc:
        with (
            tc.tile_pool(name="singles", bufs=1) as singles,
            tc.tile_pool(name="xin", bufs=3) as xin,
            tc.tile_pool(name="qp", bufs=4) as qp,
            tc.tile_pool(name="kpl", bufs=6) as kpl,
            tc.tile_pool(name="cpool", bufs=2) as cpool,
            tc.tile_pool(name="bsb", bufs=3) as bsb,
            tc.tile_pool(name="dvp", bufs=4) as dvp,
            tc.tile_pool(name="obp", bufs=3) as obp,
            # 8 psum banks: qlog 2x2 + klog 3x1 (also serves phase-D
            # tiles) + b1 1x1 = 8. Separate q/k pools let the two logit
            # pipelines (ACT-paced / DVE-paced) run concurrently.
            tc.tile_pool(name="qlog", bufs=2, space="PSUM") as qlog,
            tc.tile_pool(name="klog", bufs=3, space="PSUM") as klog,
            tc.tile_pool(name="psb1", bufs=1, space="PSUM") as psb1,
        ):
            wt_sb = singles.tile([128, M], fp16)
            nc.gpsimd.dma_start(out=wt_sb, in_=wt[:, :])

            for i in range(PPC):
                # xq first: phase A-q leads the pair. Pair-0 inputs ride the
                # gpsimd queue, which fires several us before the sync HWDGE
                # ring comes up; later pairs use sync (FIFO = need order).
                ieng = nc.gpsimd if i == 0 else nc.sync
                xq = xin.tile([128, L], fp16, tag="xq")
                xq_cuts = (0, 1024, L) if i == 0 else (0, L)
                for lo, hi in zip(xq_cuts, xq_cuts[1:]):
                    ieng.dma_start(out=xq[:, lo:hi], in_=xq_t[i, :, lo:hi])
                xk = xin.tile([128, L], fp16, tag="xk")
                for lo, hi in ((0, 2048), (2048, L)):
                    ieng.dma_start(out=xk[:, lo:hi], in_=xk_t[i, :, lo:hi])
                c_sb = cpool.tile([128, NCH, DV + 1], bf16)
                ieng.dma_start(out=c_sb, in_=cb[i])

                # ---- phase A-q: qs'^T = exp(W~ Xq~^T); g outer, mi inner
                qpT = [
                    qp.tile([128, L], bf16, tag=f"qpT{mi}", name=f"qpT{mi}_{i}")
                    for mi in range(2)
                ]
                col = 0
                for g in range(4):
                    w = 1024
                    for mi in range(2):
                        ps = qlog.tile([128, 1024], f32, tag="qlogits",
                                       name=f"psq_{i}_{mi}_{g}")
                        for n2 in range(w // 512):
                            nc.tensor.matmul(
                                ps[:, 512 * n2:512 * (n2 + 1)],
                                lhsT=wt_sb[:, 128 * mi:128 * (mi + 1)],
                                rhs=xq[:, col + 512 * n2:col + 512 * (n2 + 1)],
                                start=True, stop=True,
                            )
                        nc.scalar.activation(
                            out=qpT[mi][:, col:col + w], in_=ps[:, :w],
                            func=Exp, bias=0.0, scale=1.0,
                        )
                    col += w

                # ---- phase A-k + C: ks' natural; buf1 += ks'^T @ c.
                # Both buf1 halves accumulate in ONE psum bank: memset once,
                # then every matmul is start=False (no zero-region wipe), so
                # chunk accumulation order is free and DVE groups lead.
                b1ps = psb1.tile([128, 2, DV + 1], f32, tag="b1",
                                 name=f"b1_{i}")
                nc.vector.memset(b1ps, 0.0)
                for g, eng in enumerate(K_GROUPS):
                    ps = klog.tile([128, 512], f32, tag="klogits",
                                   name=f"psk_{i}_{g}")
                    for j in range(2):
                        n = 2 * g + j
                        nc.tensor.matmul(
                            ps[:, 256 * j:256 * (j + 1)],
                            lhsT=xk[:, 128 * n:128 * (n + 1)],
                            rhs=wt_sb,
                            start=True, stop=True,
                        )
                    kp = kpl.tile([128, 512], bf16, tag="kp", name=f"kp_{i}_{g}")
                    if eng == "act":
                        nc.scalar.activation(
                            out=kp, in_=ps, func=Exp, bias=0.0, scale=1.0)
                    else:
                        nc.vector.tensor_scalar(
                            out=kp.bitcast(i16), in0=ps,
                            scalar1=SCH_A, scalar2=SCH_B,
                            op0=mult, op1=add)
                    for j in range(2):
                        n = 2 * g + j
                        for mi in range(2):
                            nc.tensor.matmul(
                                b1ps[:, mi, :],
                                lhsT=kp[:, 256 * j + 128 * mi:256 * j + 128 * (mi + 1)],
                                rhs=c_sb[:, n, :],
                                start=False, stop=(n == NCH - 1 and mi == 1),
                                skip_group_check=True,
                            )
                buf1 = bsb.tile([128, 2, DV + 1], bf16, tag="b1sb",
                                name=f"b1sb_{i}")
                nc.scalar.copy(out=buf1, in_=b1ps)

                # ---- phase D: buf2 = qs' @ buf1 per L-chunk; divide; store
                ob = obp.tile([128, NCH * DV], fp16, tag="ob", name=f"ob_{i}")
                n0 = 0
                for gi, cnt in enumerate((7, 7, 7, 7, 4)):
                    dps = klog.tile([128, 512], f32, tag="klogits",
                                    name=f"dps_{i}_{gi}")[:, :cnt * (DV + 1)]
                    dps3 = dps.rearrange("p (s e) -> p s e", e=DV + 1)
                    for s in range(cnt):
                        n = n0 + s
                        for mi in range(2):
                            nc.tensor.matmul(
                                dps3[:, s, :],
                                lhsT=qpT[mi][:, 128 * n:128 * (n + 1)],
                                rhs=buf1[:, mi, :],
                                start=(mi == 0), stop=(mi == 1),
                            )
                    den = dvp.tile([128, 7], f32, tag="den", name=f"den_{i}_{gi}")
                    nc.vector.reciprocal(out=den[:, :cnt], in_=dps3[:, :cnt, DV])
                    den_sl = den[:, :cnt]
                    den_bc = bass.AP(
                        tensor=den_sl.tensor, offset=den_sl.offset,
                        ap=[den_sl.ap[0], den_sl.ap[1], [0, DV]])
                    nc.vector.tensor_tensor(
                        out=ob[:, DV * n0:DV * (n0 + cnt)].rearrange(
                            "p (s e) -> p s e", e=DV),
                        in0=dps3[:, :cnt, 0:DV],
                        in1=den_bc,
                        op=mybir.AluOpType.mult,
                    )
                    oeng = nc.sync if i == PPC - 1 else nc.gpsimd
                    ob3 = ob.rearrange("p (n e) -> p n e", e=DV)
                    oeng.dma_start(out=out[i, :, n0:n0 + cnt, :],
                                   in_=ob3[:, n0:n0 + cnt, :])
                    n0 += cnt
    nc.compile()
    return nc


def _prep_inputs(qs, ks, vs, qs_s, ks_s, fourier_W, qk_proj, a):
    """Host-side: fourier features, augmentation, transposes, per-core split."""
    pq = TWO_PI * (qs_s @ fourier_W)       # (B, L, 32)
    pk = TWO_PI * (ks_s @ fourier_W)
    embq = np.concatenate([np.sin(pq), np.cos(pq)], axis=-1).astype(np.float32)
    embk = np.concatenate([np.sin(pk), np.cos(pk)], axis=-1).astype(np.float32)
    qs_sp = (a * embq.reshape(B, L, H, DSH)).astype(np.float32)  # (B,L,H,8)
    ks_sp = embk.reshape(B, L, H, DSH)

    xq = np.concatenate([qs * NORM, qs_sp], axis=-1)  # (B,L,H,72)
    xk = np.concatenate([ks * NORM, ks_sp], axis=-1)
    hq = 0.5 * np.sum(np.square(xq), axis=-1, keepdims=True) + LN16
    hk = 0.5 * np.sum(np.square(xk), axis=-1, keepdims=True) + LN16
    xq = np.concatenate([xq, hq], axis=-1)  # (B,L,H,73)
    xk = np.concatenate([xk, hk], axis=-1)

    c = np.concatenate([vs, np.ones((B, L, H, 1), vs.dtype)], axis=-1)
    c = c.astype(ml_dtypes.bfloat16)        # (B,L,H,65)

    wt_f = np.concatenate(
        [qk_proj, -np.ones((M, 1), np.float32)], axis=1).T  # (73, 256)
    wt = np.zeros((128, M), np.float16)
    wt[:KAUG] = wt_f

    pairs = [(b, h) for b in range(B) for h in range(H)]
    in_maps = []
    for core in range(NCORES):
        sel = pairs[PPC * core:PPC * (core + 1)]
        xq_t = np.zeros((PPC, 128, L), np.float16)
        xk_t = np.zeros((PPC, 128, L), np.float16)
        for j, (b, h) in enumerate(sel):
            xq_t[j, :KAUG] = xq[b, :, h, :].T
            xk_t[j, :KAUG] = xk[b, :, h, :].T
        cbs = np.stack([
            c[b, :, h, :].reshape(NCH, 128, DV + 1).transpose(1, 0, 2)
            for (b, h) in sel
        ])
        in_maps.append({
            "xq_t": xq_t,
            "xk_t": xk_t,
            "cb": np.ascontiguousarray(cbs),
            "wt": wt,
        })
    return in_maps, pairs


def kernel(qs, ks, vs, qs_s, ks_s, fourier_W, qk_proj, a, _trace=False):
    from concourse.bass_utils import run_bass_kernel_spmd

    if "nc" not in _CACHE:
        _CACHE["nc"] = _build_kernel()
    nc = _CACHE["nc"]

    in_maps, pairs = _prep_inputs(
        np.asarray(qs), np.asarray(ks), np.asarray(vs), np.asarray(qs_s),
        np.asarray(ks_s), np.asarray(fourier_W), np.asarray(qk_proj),
        np.asarray(a))

    try:
        res = run_bass_kernel_spmd(
            nc, in_maps, core_ids=list(range(NCORES)), trace=_trace)
    except Exception:
        # the axon-tunneled devices occasionally throw a transient
        # NRT_EXEC_UNIT_UNRECOVERABLE; one retry has always recovered it
        res = run_bass_kernel_spmd(
            nc, in_maps, core_ids=list(range(NCORES)), trace=_trace)
    _CACHE["last_result"] = res

    full = np.empty((B, L, H, DV), np.float32)
    for core in range(NCORES):
        o = res.results[core]["out"]  # (PPC, 128, NCH, DV)
        for j, (b, h) in enumerate(pairs[PPC * core:PPC * (core + 1)]):
            full[b, :, h, :] = (
                o[j].transpose(1, 0, 2).reshape(L, DV).astype(np.float32))
    return full


# revision 20
# speedup vs baseline: 1.1074x; 1.1074x over previous
"""Distance-biased FAVOR+ fast attention on 8 Trainium2 NeuronCores.

Strategy: shard the 32 (batch, head) pairs across 8 cores (4 pairs/core).
Per pair, the device computes:
    qs'^T = exp(W~ @ Xq~^T)    (transposed layout, m on partitions)  [bf16]
    ks'   = exp(Xk~ @ W~^T)    (natural layout, L on partitions)     [bf16]
    buf1  = ks'^T @ c          (PSUM accumulation over 32 L-chunks)
    buf2  = qs' @ buf1         (natural layout), then num / den
where Xq~/Xk~ are host-precomputed 73-dim augmented features
  [x*D^-0.25 (64), fourier dist features (8), h = 0.5*|x|^2 + ln(16)]
padded to 128 rows (full-partition DMAs run 5x faster than 73-row ones)
in fp16, and W~ = [qk_proj | -1] so exp(W~ X~^T) = exp(x W^T - h)/16 = phi(x).

The exp is split across two engines: ACT does exact exp on the q side +
8 k-chunks; the DVE computes the remaining 24 k-chunks with a Schraudolph
bitcast exp (bf16 bits = int16(logit*128/ln2 + 127*128 - C)), one
tensor_scalar mult-add per group. The +-3% ripple of that approximation
averages out through buf1/buf2; measured end-to-end rel err ~1.2e-2
(gate 2e-2). The /sqrt(m) factor and the EPS clamp on den cancel /
are dead code respectively (den >= 2e-4 on this distribution).
"""

import numpy as np
import ml_dtypes

B, L, H, D, DV = 4, 4096, 8, 64, 64
M = 256
DSH = 8             # per-head fourier feature dim (S // H)
KAUG = D + DSH + 1  # 73
NCORES = 8
PPC = (B * H) // NCORES  # pairs per core = 4
NCH = L // 128           # 32 chunks of 128 rows
EPS = 1e-6
LN16 = float(np.log(16.0))
TWO_PI = 2.0 * np.pi
NORM = float(D) ** -0.25

# Schraudolph bitcast-exp constants (bf16/int16 variant)
SCH_A = 128.0 / float(np.log(2.0))
SCH_C = 9.0
SCH_B = 127.0 * 128.0 - SCH_C
# k-chunk groups of 2 (512-f32 = one psum bank each): DVE Schraudolph
# groups first (they pace buf1), exact-exp ACT groups last
K_GROUPS = (("dve",) * 13) + (("act",) * 3)

_CACHE = {}


def _build_kernel():
    import concourse.bass as bass
    import concourse.bacc as bacc
    import concourse.mybir as mybir
    import concourse.tile as tile

    f32 = mybir.dt.float32
    bf16 = mybir.dt.bfloat16
    fp16 = mybir.dt.float16
    i16 = mybir.dt.int16
    Exp = mybir.ActivationFunctionType.Exp
    mult = mybir.AluOpType.mult
    add = mybir.AluOpType.add

    nc = bacc.Bacc("TRN2", debug=False, num_devices=NCORES)
    xq_t = nc.dram_tensor("xq_t", [PPC, 128, L], fp16, kind="ExternalInput")
    xk_t = nc.dram_tensor("xk_t", [PPC, 128, L], fp16, kind="ExternalInput")
    cb = nc.dram_tensor("cb", [PPC, 128, NCH, DV + 1], bf16, kind="ExternalInput")
    wt = nc.dram_tensor("wt", [128, M], fp16, kind="ExternalInput")
    out = nc.dram_tensor("out", [PPC, 128, NCH, DV], fp16, kind="ExternalOutput")

    with tile.TileContext(nc) as tc:
        with (
            tc.tile_pool(name="singles", bufs=1) as singles,
            tc.tile_pool(name="xin", bufs=3) as xin,
            tc.tile_pool(name="qp", bufs=4) as qp,
            tc.tile_pool(name="kpl", bufs=6) as kpl,
            tc.tile_pool(name="cpool", bufs=2) as cpool,
            tc.tile_pool(name="bsb", bufs=3) as bsb,
            tc.tile_pool(name="dvp", bufs=4) as dvp,
            tc.tile_pool(name="obp", bufs=3) as obp,
            # 8 psum banks: qlog 2x2 + klog 3x1 (also serves phase-D
            # tiles) + b1 1x1 = 8. Separate q/k pools let the two logit
            # pipelines (ACT-paced / DVE-paced) run concurrently.
            tc.tile_pool(name="qlog", bufs=2, space="PSUM") as qlog,
            tc.tile_pool(name="klog", bufs=3, space="PSUM") as klog,
            tc.tile_pool(name="psb1", bufs=1, space="PSUM") as psb1,
        ):
            wt_sb = singles.tile([128, M], fp16)
            nc.sync.dma_start(out=wt_sb, in_=wt[:, :])

            for i in range(PPC):
                # xq first: phase A-q leads the pair. sync HWDGE ring is
                # FIFO, so pair order = need order.
                ieng = nc.sync
                xq = xin.tile([128, L], fp16, tag="xq")
                xq_cuts = (0, 512, 1024, L) if i == 0 else (0, L)
                for lo, hi in zip(xq_cuts, xq_cuts[1:]):
                    ieng.dma_start(out=xq[:, lo:hi], in_=xq_t[i, :, lo:hi])
                xk = xin.tile([128, L], fp16, tag="xk")
                for lo, hi in ((0, 2048), (2048, L)):
                    ieng.dma_start(out=xk[:, lo:hi], in_=xk_t[i, :, lo:hi])
                c_sb = cpool.tile([128, NCH, DV + 1], bf16)
                ieng.dma_start(out=c_sb, in_=cb[i])

                # ---- phase A-q: qs'^T = exp(W~ Xq~^T); g outer, mi inner
                qpT = [
                    qp.tile([128, L], bf16, tag=f"qpT{mi}", name=f"qpT{mi}_{i}")
                    for mi in range(2)
                ]
                col = 0
                if i == 0:
                    aq_ws = (512, 512, 1024, 1024, 1024)
                elif i == PPC - 1:
                    aq_ws = (1024, 1024, 1024, 512, 512)
                else:
                    aq_ws = (1024, 1024, 1024, 1024)
                for g, w in enumerate(aq_ws):
                    for mi in range(2):
                        ps = qlog.tile([128, 1024], f32, tag="qlogits",
                                       name=f"psq_{i}_{mi}_{g}")
                        for n2 in range(w // 512):
                            nc.tensor.matmul(
                                ps[:, 512 * n2:512 * (n2 + 1)],
                                lhsT=wt_sb[:, 128 * mi:128 * (mi + 1)],
                                rhs=xq[:, col + 512 * n2:col + 512 * (n2 + 1)],
                                start=True, stop=True,
                            )
                        nc.scalar.activation(
                            out=qpT[mi][:, col:col + w], in_=ps[:, :w],
                            func=Exp, bias=0.0, scale=1.0,
                        )
                    col += w

                # ---- phase A-k + C: ks' natural; buf1 += ks'^T @ c.
                # Both buf1 halves accumulate in ONE psum bank: memset once,
                # then every matmul is start=False (no zero-region wipe), so
                # chunk accumulation order is free and DVE groups lead.
                b1ps = psb1.tile([128, 2, DV + 1], f32, tag="b1",
                                 name=f"b1_{i}")
                nc.vector.memset(b1ps, 0.0)
                for g, eng in enumerate(K_GROUPS):
                    ps = klog.tile([128, 512], f32, tag="klogits",
                                   name=f"psk_{i}_{g}")
                    for j in range(2):
                        n = 2 * g + j
                        nc.tensor.matmul(
                            ps[:, 256 * j:256 * (j + 1)],
                            lhsT=xk[:, 128 * n:128 * (n + 1)],
                            rhs=wt_sb,
                            start=True, stop=True,
                        )
                    kp = kpl.tile([128, 512], bf16, tag="kp", name=f"kp_{i}_{g}")
                    if eng == "act":
                        nc.scalar.activation(
                            out=kp, in_=ps, func=Exp, bias=0.0, scale=1.0)
                    else:
                        nc.vector.tensor_scalar(
                            out=kp.bitcast(i16), in0=ps,
                            scalar1=SCH_A, scalar2=SCH_B,
                            op0=mult, op1=add)
                    for j in range(2):
                        n = 2 * g + j
                        for mi in range(2):
                            nc.tensor.matmul(
                                b1ps[:, mi, :],
                                lhsT=kp[:, 256 * j + 128 * mi:256 * j + 128 * (mi + 1)],
                                rhs=c_sb[:, n, :],
                                start=False, stop=(n == NCH - 1 and mi == 1),
                                skip_group_check=True,
                            )
                buf1 = bsb.tile([128, 2, DV + 1], bf16, tag="b1sb",
                                name=f"b1sb_{i}")
                nc.scalar.copy(out=buf1, in_=b1ps)

                # ---- phase D: buf2 = qs' @ buf1 per L-chunk; divide; store
                ob = obp.tile([128, NCH * DV], fp16, tag="ob", name=f"ob_{i}")
                n0 = 0
                for gi, cnt in enumerate((7, 7, 7, 7, 4)):
                    dps = klog.tile([128, 512], f32, tag="klogits",
                                    name=f"dps_{i}_{gi}")[:, :cnt * (DV + 1)]
                    dps3 = dps.rearrange("p (s e) -> p s e", e=DV + 1)
                    for s in range(cnt):
                        n = n0 + s
                        for mi in range(2):
                            nc.tensor.matmul(
                                dps3[:, s, :],
                                lhsT=qpT[mi][:, 128 * n:128 * (n + 1)],
                                rhs=buf1[:, mi, :],
                                start=(mi == 0), stop=(mi == 1),
                            )
                    den = dvp.tile([128, 7], f32, tag="den", name=f"den_{i}_{gi}")
                    nc.vector.reciprocal(out=den[:, :cnt], in_=dps3[:, :cnt, DV])
                    den_sl = den[:, :cnt]
                    den_bc = bass.AP(
                        tensor=den_sl.tensor, offset=den_sl.offset,
                        ap=[den_sl.ap[0], den_sl.ap[1], [0, DV]])
                    nc.vector.tensor_tensor(
                        out=ob[:, DV * n0:DV * (n0 + cnt)].rearrange(
                            "p (s e) -> p s e", e=DV),
                        in0=dps3[:, :cnt, 0:DV],
                        in1=den_bc,
                        op=mybir.AluOpType.mult,
                    )
                    oeng = nc.sync if i == PPC - 1 else nc.gpsimd
                    ob3 = ob.rearrange("p (n e) -> p n e", e=DV)
                    oeng.dma_start(out=out[i, :, n0:n0 + cnt, :],
                                   in_=ob3[:, n0:n0 + cnt, :])
                    n0 += cnt
    nc.compile()
    return nc


def _prep_inputs(qs, ks, vs, qs_s, ks_s, fourier_W, qk_proj, a):
    """Host-side: fourier features, augmentation, transposes, per-core split."""
    pq = TWO_PI * (qs_s @ fourier_W)       # (B, L, 32)
    pk = TWO_PI * (ks_s @ fourier_W)
    embq = np.concatenate([np.sin(pq), np.cos(pq)], axis=-1).astype(np.float32)
    embk = np.concatenate([np.sin(pk), np.cos(pk)], axis=-1).astype(np.float32)
    qs_sp = (a * embq.reshape(B, L, H, DSH)).astype(np.float32)  # (B,L,H,8)
    ks_sp = embk.reshape(B, L, H, DSH)

    xq = np.concatenate([qs * NORM, qs_sp], axis=-1)  # (B,L,H,72)
    xk = np.concatenate([ks * NORM, ks_sp], axis=-1)
    hq = 0.5 * np.sum(np.square(xq), axis=-1, keepdims=True) + LN16
    hk = 0.5 * np.sum(np.square(xk), axis=-1, keepdims=True) + LN16
    xq = np.concatenate([xq, hq], axis=-1)  # (B,L,H,73)
    xk = np.concatenate([xk, hk], axis=-1)

    c = np.concatenate([vs, np.ones((B, L, H, 1), vs.dtype)], axis=-1)
    c = c.astype(ml_dtypes.bfloat16)        # (B,L,H,65)

    wt_f = np.concatenate(
        [qk_proj, -np.ones((M, 1), np.float32)], axis=1).T  # (73, 256)
    wt = np.zeros((128, M), np.float16)
    wt[:KAUG] = wt_f

    pairs = [(b, h) for b in range(B) for h in range(H)]
    in_maps = []
    for core in range(NCORES):
        sel = pairs[PPC * core:PPC * (core + 1)]
        xq_t = np.zeros((PPC, 128, L), np.float16)
        xk_t = np.zeros((PPC, 128, L), np.float16)
        for j, (b, h) in enumerate(sel):
            xq_t[j, :KAUG] = xq[b, :, h, :].T
            xk_t[j, :KAUG] = xk[b, :, h, :].T
        cbs = np.stack([
            c[b, :, h, :].reshape(NCH, 128, DV + 1).transpose(1, 0, 2)
            for (b, h) in sel
        ])
        in_maps.append({
            "xq_t": xq_t,
            "xk_t": xk_t,
            "cb": np.ascontiguousarray(cbs),
            "wt": wt,
        })
    return in_maps, pairs


def kernel(qs, ks, vs, qs_s, ks_s, fourier_W, qk_proj, a, _trace=False):
    from concourse.bass_utils import run_bass_kernel_spmd

    if "nc" not in _CACHE:
        _CACHE["nc"] = _build_kernel()
    nc = _CACHE["nc"]

    in_maps, pairs = _prep_inputs(
        np.asarray(qs), np.asarray(ks), np.asarray(vs), np.asarray(qs_s),
        np.asarray(ks_s), np.asarray(fourier_W), np.asarray(qk_proj),
        np.asarray(a))

    try:
        res = run_bass_kernel_spmd(
            nc, in_maps, core_ids=list(range(NCORES)), trace=_trace)
    except Exception:
        # the axon-tunneled devices occasionally throw a transient
        # NRT_EXEC_UNIT_UNRECOVERABLE; one retry has always recovered it
        res = run_bass_kernel_spmd(
            nc, in_maps, core_ids=list(range(NCORES)), trace=_trace)
    _CACHE["last_result"] = res

    full = np.empty((B, L, H, DV), np.float32)
    for core in range(NCORES):
        o = res.results[core]["out"]  # (PPC, 128, NCH, DV)
        for j, (b, h) in enumerate(pairs[PPC * core:PPC * (core + 1)]):
            full[b, :, h, :] = (
                o[j].transpose(1, 0, 2).reshape(L, DV).astype(np.float32))
    return full


# revision 44
# speedup vs baseline: 1.1814x; 1.0669x over previous
"""Distance-biased FAVOR+ fast attention on 8 Trainium2 NeuronCores.

Strategy: shard the 32 (batch, head) pairs across 8 cores (4 pairs/core).
Per pair, the device computes:
    qs'^T = exp(W~ @ Xq~^T)    (transposed layout, m on partitions)  [bf16]
    ks'   = exp(Xk~ @ W~^T)    (natural layout, L on partitions)     [bf16]
    buf1  = ks'^T @ c          (PSUM accumulation over 32 L-chunks)
    buf2  = qs' @ buf1         (natural layout), then num / den
where Xq~/Xk~ are host-precomputed 73-dim augmented features
  [x*D^-0.25 (64), fourier dist features (8), h = 0.5*|x|^2 + ln(16)]
padded to 128 rows (full-partition DMAs run 5x faster than 73-row ones)
in fp16, and W~ = [qk_proj | -1] so exp(W~ X~^T) = exp(x W^T - h)/16 = phi(x).

The exp is split across two engines: ACT does exact exp on the q side +
8 k-chunks; the DVE computes the remaining 24 k-chunks with a Schraudolph
bitcast exp (bf16 bits = int16(logit*128/ln2 + 127*128 - C)), one
tensor_scalar mult-add per group. The +-3% ripple of that approximation
averages out through buf1/buf2; measured end-to-end rel err ~1.2e-2
(gate 2e-2). The /sqrt(m) factor and the EPS clamp on den cancel /
are dead code respectively (den >= 2e-4 on this distribution).
"""

import numpy as np
import ml_dtypes

B, L, H, D, DV = 4, 4096, 8, 64, 64
M = 256
DSH = 8             # per-head fourier feature dim (S // H)
KAUG = D + DSH + 1  # 73
NCORES = 8
PPC = (B * H) // NCORES  # pairs per core = 4
NCH = L // 128           # 32 chunks of 128 rows
EPS = 1e-6
LN16 = float(np.log(16.0))
TWO_PI = 2.0 * np.pi
NORM = float(D) ** -0.25

# Schraudolph bitcast-exp constants (bf16/int16 variant)
SCH_A = 128.0 / float(np.log(2.0))
SCH_C = 9.0
SCH_B = 127.0 * 128.0 - SCH_C
# k-chunk groups of 2 (512-f32 = one psum bank each): DVE Schraudolph
# groups first (they pace buf1), exact-exp ACT groups last
K_GROUPS = (("dve",) * 14) + (("act",) * 2)

_CACHE = {}


def _build_kernel():
    import concourse.bass as bass
    import concourse.bacc as bacc
    import concourse.mybir as mybir
    import concourse.tile as tile

    f32 = mybir.dt.float32
    bf16 = mybir.dt.bfloat16
    fp16 = mybir.dt.float16
    i16 = mybir.dt.int16
    Exp = mybir.ActivationFunctionType.Exp
    mult = mybir.AluOpType.mult
    add = mybir.AluOpType.add

    nc = bacc.Bacc("TRN2", debug=False, num_devices=NCORES)
    xq_t = nc.dram_tensor("xq_t", [PPC, 128, L], fp16, kind="ExternalInput")
    xk_t = nc.dram_tensor("xk_t", [PPC, 128, L], fp16, kind="ExternalInput")
    cb = nc.dram_tensor("cb", [PPC, 128, NCH, DV + 1], bf16, kind="ExternalInput")
    wt = nc.dram_tensor("wt", [128, M], fp16, kind="ExternalInput")
    # raw buf2 (num | den) in f32; the num/den division happens on host
    out = nc.dram_tensor("out", [PPC, 128, NCH, DV + 1], f32,
                         kind="ExternalOutput")

    with tile.TileContext(nc) as tc:
        with (
            tc.tile_pool(name="singles", bufs=1) as singles,
            tc.tile_pool(name="xin", bufs=3) as xin,
            tc.tile_pool(name="qp", bufs=4) as qp,
            tc.tile_pool(name="kpl", bufs=6) as kpl,
            tc.tile_pool(name="cpool", bufs=2) as cpool,
            tc.tile_pool(name="bsb", bufs=3) as bsb,
            tc.tile_pool(name="obp", bufs=3) as obp,
            # 8 psum banks: qlog 2x2 + klog 3x1 (also serves phase-D
            # tiles) + b1 1x1 = 8. Separate q/k pools let the two logit
            # pipelines (ACT-paced / DVE-paced) run concurrently.
            tc.tile_pool(name="qlog", bufs=2, space="PSUM") as qlog,
            tc.tile_pool(name="klog", bufs=3, space="PSUM") as klog,
            tc.tile_pool(name="psb1", bufs=1, space="PSUM") as psb1,
        ):
            # wt rides the gpsimd ring: lands in parallel with the sync
            # ring's first xq piece instead of serializing ahead of it
            wt_sb = singles.tile([128, M], fp16)
            nc.gpsimd.dma_start(out=wt_sb, in_=wt[:, :])

            # PE warm-up: dummy matmuls on a zeroed scratch tile span the
            # input-DMA wait so the p-state ramp completes before real
            # work lands (cold 512-col matmuls otherwise run ~2x slow)
            scratch = singles.tile([128, 512], fp16, name="warm_src")
            nc.gpsimd.memset(scratch, 0.0)
            warm_ps = klog.tile([128, 512], f32, tag="klogits", name="warm")
            for _ in range(18):
                nc.tensor.matmul(warm_ps, lhsT=scratch[:, :128],
                                 rhs=scratch, start=True, stop=True)



            # phase-D emission of pair i is deferred into pair i+1's
            # section (interleaved between its q-groups): the late-
            # dependency copies then never sit at an engine queue's pair
            # boundary blocking the next pair's exp stream.
            pending_d = None

            for i in range(PPC):
                # xq first: phase A-q leads the pair. sync HWDGE ring is
                # FIFO, so pair order = need order.
                ieng = nc.sync
                xq = xin.tile([128, L], fp16, tag="xq")
                xq_cuts = (0, 1024, L) if i == 0 else (0, L)
                for lo, hi in zip(xq_cuts, xq_cuts[1:]):
                    ieng.dma_start(out=xq[:, lo:hi], in_=xq_t[i, :, lo:hi])
                xk = xin.tile([128, L], fp16, tag="xk")
                for lo, hi in ((0, 2048), (2048, L)):
                    ieng.dma_start(out=xk[:, lo:hi], in_=xk_t[i, :, lo:hi])
                c_sb = cpool.tile([128, NCH, DV + 1], bf16)
                ieng.dma_start(out=c_sb, in_=cb[i])

                # ---- phases A-q and A-k+C, emission interleaved so the
                # Tensor queue alternates q-logit groups with k-logit
                # groups: the DVE Schraudolph chain (which paces buf1)
                # starts right at pair start instead of after all q-mms.
                # buf1 halves accumulate in ONE psum bank: memset once,
                # then every matmul is start=False (no zero-region wipe),
                # so chunk accumulation order is free.
                qpT = [
                    qp.tile([128, L], bf16, tag=f"qpT{mi}", name=f"qpT{mi}_{i}")
                    for mi in range(2)
                ]
                b1ps = psb1.tile([128, 2, DV + 1], f32, tag="b1",
                                 name=f"b1_{i}")
                nc.vector.memset(b1ps, 0.0)

                def q_group(g, col, w):
                    for mi in range(2):
                        ps = qlog.tile([128, 1024], f32, tag="qlogits",
                                       name=f"psq_{i}_{mi}_{g}")
                        for n2 in range(w // 512):
                            nc.tensor.matmul(
                                ps[:, 512 * n2:512 * (n2 + 1)],
                                lhsT=wt_sb[:, 128 * mi:128 * (mi + 1)],
                                rhs=xq[:, col + 512 * n2:col + 512 * (n2 + 1)],
                                start=True, stop=True,
                            )
                        nc.scalar.activation(
                            out=qpT[mi][:, col:col + w], in_=ps[:, :w],
                            func=Exp, bias=0.0, scale=1.0,
                        )

                def k_group(g, eng):
                    ps = klog.tile([128, 512], f32, tag="klogits",
                                   name=f"psk_{i}_{g}")
                    for j in range(2):
                        n = 2 * g + j
                        nc.tensor.matmul(
                            ps[:, 256 * j:256 * (j + 1)],
                            lhsT=xk[:, 128 * n:128 * (n + 1)],
                            rhs=wt_sb,
                            start=True, stop=True,
                        )
                    kp = kpl.tile([128, 512], bf16, tag="kp", name=f"kp_{i}_{g}")
                    if eng == "act":
                        nc.scalar.activation(
                            out=kp, in_=ps, func=Exp, bias=0.0, scale=1.0)
                    else:
                        nc.vector.tensor_scalar(
                            out=kp.bitcast(i16), in0=ps,
                            scalar1=SCH_A, scalar2=SCH_B,
                            op0=mult, op1=add)
                    for j in range(2):
                        n = 2 * g + j
                        for mi in range(2):
                            nc.tensor.matmul(
                                b1ps[:, mi, :],
                                lhsT=kp[:, 256 * j + 128 * mi:256 * j + 128 * (mi + 1)],
                                rhs=c_sb[:, n, :],
                                start=False, stop=(n == NCH - 1 and mi == 1),
                                skip_group_check=True,
                            )

                def make_d_group(io, qpT_o, buf1_o, ob_o, ob3_o, gi, n0, cnt):
                    def emit():
                        dps = klog.tile([128, 512], f32, tag="klogits",
                                        name=f"dps_{io}_{gi}")[:, :cnt * (DV + 1)]
                        dps3 = dps.rearrange("p (s e) -> p s e", e=DV + 1)
                        for s in range(cnt):
                            n = n0 + s
                            for mi in range(2):
                                nc.tensor.matmul(
                                    dps3[:, s, :],
                                    lhsT=qpT_o[mi][:, 128 * n:128 * (n + 1)],
                                    rhs=buf1_o[:, mi, :],
                                    start=(mi == 0), stop=(mi == 1),
                                )
                        if gi in (0, 2):
                            nc.scalar.copy(
                                out=ob_o[:, (DV + 1) * n0:(DV + 1) * (n0 + cnt)],
                                in_=dps)
                        else:
                            nc.vector.tensor_copy(
                                out=ob_o[:, (DV + 1) * n0:(DV + 1) * (n0 + cnt)],
                                in_=dps)
                        oeng = nc.sync if io == PPC - 1 else nc.gpsimd
                        oeng.dma_start(out=out[io, :, n0:n0 + cnt, :],
                                       in_=ob3_o[:, n0:n0 + cnt, :])
                    return emit

                # A-phase emission with the previous pair's D-groups woven in
                dq = list(pending_d or [])
                for g in range(4):
                    q_group(g, 1024 * g, 1024)
                    if dq:
                        dq.pop(0)()
                    if dq:
                        dq.pop(0)()
                    for kg in range(4 * g, 4 * g + 4):
                        k_group(kg, K_GROUPS[kg])
                while dq:
                    dq.pop(0)()
                buf1 = bsb.tile([128, 2, DV + 1], bf16, tag="b1sb",
                                name=f"b1sb_{i}")
                nc.vector.tensor_copy(out=buf1, in_=b1ps)

                # ---- phase D: buf2 = qs' @ buf1; copy psum -> sbuf f32
                # (split ACT/DVE); store raw num|den, divide on host
                ob = obp.tile([128, NCH * (DV + 1)], f32, tag="ob",
                              name=f"ob_{i}")
                ob3 = ob.rearrange("p (n e) -> p n e", e=DV + 1)
                dcnts = (7, 7, 7, 7, 2, 2) if i == PPC - 1 else (7, 7, 7, 7, 4)
                pending_d = []
                n0 = 0
                for gi, cnt in enumerate(dcnts):
                    pending_d.append(
                        make_d_group(i, qpT, buf1, ob, ob3, gi, n0, cnt))
                    n0 += cnt
                if i == PPC - 1:
                    for emit in pending_d:
                        emit()
    nc.compile()
    return nc


def _prep_inputs(qs, ks, vs, qs_s, ks_s, fourier_W, qk_proj, a):
    """Host-side: fourier features, augmentation, transposes, per-core split."""
    pq = TWO_PI * (qs_s @ fourier_W)       # (B, L, 32)
    pk = TWO_PI * (ks_s @ fourier_W)
    embq = np.concatenate([np.sin(pq), np.cos(pq)], axis=-1).astype(np.float32)
    embk = np.concatenate([np.sin(pk), np.cos(pk)], axis=-1).astype(np.float32)
    qs_sp = (a * embq.reshape(B, L, H, DSH)).astype(np.float32)  # (B,L,H,8)
    ks_sp = embk.reshape(B, L, H, DSH)

    xq = np.concatenate([qs * NORM, qs_sp], axis=-1)  # (B,L,H,72)
    xk = np.concatenate([ks * NORM, ks_sp], axis=-1)
    hq = 0.5 * np.sum(np.square(xq), axis=-1, keepdims=True) + LN16
    hk = 0.5 * np.sum(np.square(xk), axis=-1, keepdims=True) + LN16
    xq = np.concatenate([xq, hq], axis=-1)  # (B,L,H,73)
    xk = np.concatenate([xk, hk], axis=-1)

    c = np.concatenate([vs, np.ones((B, L, H, 1), vs.dtype)], axis=-1)
    c = c.astype(ml_dtypes.bfloat16)        # (B,L,H,65)

    wt_f = np.concatenate(
        [qk_proj, -np.ones((M, 1), np.float32)], axis=1).T  # (73, 256)
    wt = np.zeros((128, M), np.float16)
    wt[:KAUG] = wt_f

    pairs = [(b, h) for b in range(B) for h in range(H)]
    in_maps = []
    for core in range(NCORES):
        sel = pairs[PPC * core:PPC * (core + 1)]
        xq_t = np.zeros((PPC, 128, L), np.float16)
        xk_t = np.zeros((PPC, 128, L), np.float16)
        for j, (b, h) in enumerate(sel):
            xq_t[j, :KAUG] = xq[b, :, h, :].T
            xk_t[j, :KAUG] = xk[b, :, h, :].T
        cbs = np.stack([
            c[b, :, h, :].reshape(NCH, 128, DV + 1).transpose(1, 0, 2)
            for (b, h) in sel
        ])
        in_maps.append({
            "xq_t": xq_t,
            "xk_t": xk_t,
            "cb": np.ascontiguousarray(cbs),
            "wt": wt,
        })
    return in_maps, pairs


def kernel(qs, ks, vs, qs_s, ks_s, fourier_W, qk_proj, a, _trace=False):
    from concourse.bass_utils import run_bass_kernel_spmd

    if "nc" not in _CACHE:
        _CACHE["nc"] = _build_kernel()
    nc = _CACHE["nc"]

    in_maps, pairs = _prep_inputs(
        np.asarray(qs), np.asarray(ks), np.asarray(vs), np.asarray(qs_s),
        np.asarray(ks_s), np.asarray(fourier_W), np.asarray(qk_proj),
        np.asarray(a))

    try:
        res = run_bass_kernel_spmd(
            nc, in_maps, core_ids=list(range(NCORES)), trace=_trace)
    except Exception:
        # the axon-tunneled devices occasionally throw a transient
        # NRT_EXEC_UNIT_UNRECOVERABLE; one retry has always recovered it
        res = run_bass_kernel_spmd(
            nc, in_maps, core_ids=list(range(NCORES)), trace=_trace)
    _CACHE["last_result"] = res

    full = np.empty((B, L, H, DV), np.float32)
    for core in range(NCORES):
        o = res.results[core]["out"]  # (PPC, 128, NCH, DV+1) f32 raw buf2
        for j, (b, h) in enumerate(pairs[PPC * core:PPC * (core + 1)]):
            buf2 = o[j].transpose(1, 0, 2).reshape(L, DV + 1)
            den = buf2[:, DV:]
            den = np.where(den < EPS, EPS, den)
            full[b, :, h, :] = buf2[:, :DV] / den
    return full
